# revision 1
# baseline (speedup 1.0000x reference)
"""Trainium2 Bass kernel for CausalSelfAttention (RoPE + GQA), 8-core SPMD.

Sharding: 8 cores = 4 batches x 2 query-halves. Each core owns four
query-256-blocks paired {i, 7-i} so causal work is balanced. Keys are
PERMUTED per core: block order = [own q-blocks (desc causal depth), then
remaining blocks ascending]. With that order, slot s consumes exactly the
static key-chunk range [2s, 2s+PAD_s), its diagonal chunks are 2s..2s+1
(emitted last so one bf16 mask multiply per slot covers diag+pad), and the
first 1024 key columns ARE the core's queries -- so Q-projection re-reads
the same xkT input and the K RoPE tables double as Q tables. Every core
runs an identical instruction stream; all variation is input data.

Device pipeline per core:
  QKV projections in fp32r (inputs pre-rounded on host, DMA'd straight into
  f32r tiles -- no cast ops); RoPE via double projection (normal +
  pair-swapped weights on PE) + two table multiplies (DVE) + adds (GPSIMD).
  Attention per (head, slot): S^T = K^T.T @ Q^T in fp32r with keys on
  partitions, exp on ScalarE (PSUM -> bf16, scale=1/8), one bf16 mask
  multiply, P.V as bf16 matmul with a ones-augmented V column yielding the
  softmax denominator free, reciprocal + gpsimd partition-broadcast divide.
  fp32r output projection interleaved with the last attention slots.
  PSUM banks are partitioned so attention overlaps the projection phases.
"""
import sys

sys.path.insert(0, "/opt/trn_rl_repo")

import numpy as np
import ml_dtypes

B, T, C = 4, 2048, 576
H, HKV, D = 9, 3, 64
THETA = 10000.0
QB = 256                      # query block
TQ = 1024                     # queries per core
SLOT_PAD = [16, 12, 8, 4]     # padded key-chunk counts per slot
QBLOCKS = [[7, 5, 2, 0], [6, 4, 3, 1]]   # q-256-block ids per half j
KEYORDER = [[7, 5, 2, 0, 1, 3, 4, 6], [6, 4, 3, 1, 0, 2, 5, 7]]
CCX = [(0, 128), (128, 128), (256, 128), (384, 128), (512, 65)]   # x chunks (577 rows incl ones)
CCQ = [(0, 128), (128, 128), (256, 128), (384, 128), (512, 64)]   # 576-row chunks
MM = [(0, 128), (128, 128), (256, 128), (384, 128), (512, 64)]    # output-dim chunks of 576


def _slot_seq(s):
    """Key-chunk emission order for slot s: fulls, then the two diag chunks."""
    return list(range(2 * s + 2, 2 * s + SLOT_PAD[s])) + [2 * s, 2 * s + 1]


_PROG = None


def _rne12(x):
    """Round fp32 to f32r (RNE, drop 12 mantissa bits) -- matches TRN2."""
    b = np.ascontiguousarray(x, np.float32).view(np.uint32).astype(np.uint64)
    lsb = (b >> np.uint64(12)) & np.uint64(1)
    r = (b + np.uint64(2047) + lsb) >> np.uint64(12) << np.uint64(12)
    return (r & np.uint64(0xFFFFFFFF)).astype(np.uint32).view(np.float32)


def _build_program(ablate=(), reps=1):
    import concourse.bacc as bacc
    import concourse.mybir as mybir
    import concourse.tile as tile

    dt = mybir.dt
    f32, f32r, bf16 = dt.float32, dt.float32r, dt.bfloat16
    AF = mybir.ActivationFunctionType

    nc = bacc.Bacc("TRN2", target_bir_lowering=False, debug=False, num_devices=8)

    def inp(name, shape, d=f32):
        return nc.declare_dram_parameter(name, shape, d, isOutput=False)

    xkT = inp("xkT", [577, T], f32r)
    wqT = inp("wqT", [C, C], f32r)
    wqsT = inp("wqsT", [C, C], f32r)
    wkT = inp("wkT", [C, HKV * D], f32r)
    wksT = inp("wksT", [C, HKV * D], f32r)
    wvT = inp("wvT", [577, 260], f32r)
    woT = inp("woT", [C, C], f32r)
    c2k = inp("c2k", [128, T])
    s2k = inp("s2k", [128, T])
    masksp = inp("masks", [16 * 128, QB], bf16)
    yT = nc.declare_dram_parameter("yT", [C, TQ], f32, isOutput=True)

    with tile.TileContext(nc) as tc:
      for _rep in range(reps):
            with (
                tc.tile_pool(name="const", bufs=1) as cp,
                tc.tile_pool(name="tab", bufs=1) as tab,
                tc.tile_pool(name="rope", bufs=2) as rp,
                tc.tile_pool(name="pwork", bufs=3) as pw,
                # attention PSUM lives on banks disjoint from the phase pools so
                # attention can overlap the projections
                tc.tile_pool(name="psS", bufs=2, space="PSUM") as psS,
                tc.tile_pool(name="psY", bufs=2, space="PSUM") as psY,
            ):
                def load_w(pool, param, chunks, cols, tag):
                    tiles = []
                    for i, (k0, kl) in enumerate(chunks):
                        t = pool.tile([128, cols], f32r, tag=f"{tag}{i}", name=f"{tag}{i}")
                        nc.sync.dma_start(t[:kl, :], param[k0:k0 + kl, :])
                        tiles.append(t)
                    return tiles

                wo_r = load_w(cp, woT, MM, C, "wo")
                m_b = cp.tile([128, 16 * QB], bf16, tag="masks", name="masks")
                for i in range(16):
                    nc.sync.dma_start(m_b[:, i * QB:(i + 1) * QB],
                                      masksp[i * 128:(i + 1) * 128, :])
                c2k_t = tab.tile([128, T], f32, tag="c2k", name="c2k")
                s2k_t = tab.tile([128, T], f32, tag="s2k", name="s2k")
                nc.sync.dma_start(c2k_t[:], c2k[:])
                nc.sync.dma_start(s2k_t[:], s2k[:])

                # persistent projection outputs
                kt_h = [cp.tile([64, T], f32r, tag=f"kt{g}", name=f"kt{g}")
                        for g in range(HKV)]
                qth = [cp.tile([64, TQ], f32r, tag=f"qth{h}", name=f"qth{h}")
                       for h in range(H)]
                v_t = [cp.tile([128, 260], f32r, tag=f"v{c}", name=f"v{c}")
                       for c in range(16)]
                ypr = [cp.tile([128, TQ], f32r, tag=f"ypr{p}", name=f"ypr{p}")
                       for p in range(5)]

                def rope(ps, pssw, rows, cols0, n, dsts):
                    """dsts[bi][:, cols0:+n] = ps*c2 + pssw*s2, per 64-row block."""
                    t1 = rp.tile([128, 512], f32r, tag="rope1", name="rope1")
                    t2 = rp.tile([128, 512], f32r, tag="rope2", name="rope2")
                    nc.vector.tensor_mul(t1[:rows, :n], ps[:rows, :n],
                                         c2k_t[:rows, cols0:cols0 + n])
                    nc.vector.tensor_mul(t2[:rows, :n], pssw[:rows, :n],
                                         s2k_t[:rows, cols0:cols0 + n])
                    for bi, dt_ in enumerate(dsts):
                        nc.gpsimd.tensor_add(dt_[0:64, cols0:cols0 + n],
                                             t1[64 * bi:64 * bi + 64, :n],
                                             t2[64 * bi:64 * bi + 64, :n])

                # ---------- phase 1: K-proj+rope, V-proj (keys, 4 windows) -----
                if "phase1" not in ablate:
                  with (
                      tc.tile_pool(name="wkv", bufs=1) as wkvp,
                      tc.tile_pool(name="psA", bufs=1, space="PSUM") as psA,
                      tc.tile_pool(name="psB", bufs=1, space="PSUM") as psB,
                  ):
                    wk_r = load_w(wkvp, wkT, CCQ, HKV * D, "wk")
                    wks_r = load_w(wkvp, wksT, CCQ, HKV * D, "wks")
                    wv_r = load_w(wkvp, wvT, CCX, 260, "wv")
                    with tc.tile_pool(name="xk", bufs=2) as xkp:
                        for nn_ in range(4):
                            xk_r = []
                            for i, (k0, kl) in enumerate(CCX):
                                t = xkp.tile([128, 512], f32r, tag=f"xk{i}",
                                             name=f"xk{i}")
                                nc.sync.dma_start(
                                    t[:kl, :],
                                    xkT[k0:k0 + kl, 512 * nn_:512 * (nn_ + 1)])
                                xk_r.append(t)

                            for mi, (mc0, mrows) in enumerate([(0, 128), (128, 64)]):
                                ps = psA.tile([128, 512], f32, tag="pja", name="pja")
                                pss = psB.tile([128, 512], f32, tag="pjb", name="pjb")
                                for ci, (k0, kl) in enumerate(CCQ):
                                    nc.tensor.matmul(
                                        ps[:mrows, :],
                                        wk_r[ci][:kl, mc0:mc0 + mrows],
                                        xk_r[ci][:kl, :],
                                        start=(ci == 0), stop=(ci == 4))
                                for ci, (k0, kl) in enumerate(CCQ):
                                    nc.tensor.matmul(
                                        pss[:mrows, :],
                                        wks_r[ci][:kl, mc0:mc0 + mrows],
                                        xk_r[ci][:kl, :],
                                        start=(ci == 0), stop=(ci == 4))
                                rope(ps, pss, mrows, 512 * nn_, 512,
                                     [kt_h[0], kt_h[1]] if mi == 0 else [kt_h[2]])

                            for ti in range(4):
                                t_ = 4 * nn_ + ti
                                vpool = psA if ti % 2 == 0 else psB
                                vtag = "pja" if ti % 2 == 0 else "pjb"
                                ps = vpool.tile([128, 512], f32, tag=vtag, name=vtag)
                                for ci, (k0, kl) in enumerate(CCX):
                                    nc.tensor.matmul(
                                        ps[:, :260],
                                        xk_r[ci][:kl, 128 * ti:128 * (ti + 1)],
                                        wv_r[ci][:kl, :],
                                        start=(ci == 0), stop=(ci == 4))
                                nc.scalar.activation(v_t[t_][:], ps[:, :260], AF.Copy)

                # ---------- attention helpers ------------------------------
                def attn_slot(s):
                    seq = _slot_seq(s)
                    n = len(seq)
                    for h in range(H):
                        g = h // 3
                        hp, hr = h // 2, 64 * (h % 2)
                        y_ps = psY.tile([65, QB], f32, tag="ypsum", name="ypsum")
                        for sc in range(n // 4):
                            sp = psS.tile([128, 4 * QB], f32, tag="scores",
                                          name="scores")
                            for i in range(4):
                                c = seq[4 * sc + i]
                                nc.tensor.matmul(
                                    sp[:, QB * i:QB * (i + 1)],
                                    kt_h[g][0:64, 128 * c:128 * (c + 1)],
                                    qth[h][0:64, QB * s:QB * (s + 1)],
                                    start=True, stop=True)
                            p_b = pw.tile([128, 4 * QB], f32r, tag="p", name="p")
                            nc.scalar.activation(p_b[:], sp[:], AF.Exp, scale=0.125)
                            if sc == n // 4 - 1:
                                nc.vector.tensor_mul(
                                    p_b[:], p_b[:],
                                    m_b[:, 1024 * s:1024 * (s + 1)])
                            for i in range(4):
                                c = seq[4 * sc + i]
                                nc.tensor.matmul(
                                    y_ps[:], v_t[c][:, 65 * g:65 * g + 65],
                                    p_b[:, QB * i:QB * (i + 1)],
                                    start=(4 * sc + i == 0),
                                    stop=(4 * sc + i == n - 1))
                        recip = pw.tile([1, QB], f32, tag="recip", name="recip")
                        nc.vector.reciprocal(recip[:], y_ps[64:65, :])
                        rb_sb = pw.tile([D, QB], f32, tag="rb", name="rb")
                        nc.gpsimd.partition_broadcast(rb_sb[:], recip[:], D)
                        nc.vector.tensor_mul(
                            ypr[hp][hr:hr + 64, QB * s:QB * (s + 1)],
                            y_ps[0:64, :], rb_sb[:])

                # ---------- phase 2 (Q-proj) interleaved with attention --------
                if "phase2" not in ablate:
                  with (
                      tc.tile_pool(name="wq", bufs=1) as wqp,
                      tc.tile_pool(name="psA2", bufs=1, space="PSUM") as psA,
                      tc.tile_pool(name="psB2", bufs=1, space="PSUM") as psB,
                  ):
                    wq_r = load_w(wqp, wqT, CCQ, C, "wq")
                    wqs_r = load_w(wqp, wqsT, CCQ, C, "wqs")
                    with tc.tile_pool(name="xq", bufs=2) as xqp:
                        for nn_ in (1, 0):      # window 1 feeds slots 3,2 (run first)
                            xq_r = []
                            for i, (k0, kl) in enumerate(CCQ):
                                t = xqp.tile([128, 512], f32r, tag=f"xq{i}",
                                             name=f"xq{i}")
                                nc.sync.dma_start(
                                    t[:kl, :],
                                    xkT[k0:k0 + kl, 512 * nn_:512 * (nn_ + 1)])
                                xq_r.append(t)

                            for m, (mc0, mrows) in enumerate(MM):
                                ps = psA.tile([128, 512], f32, tag="pja", name="pja")
                                pss = psB.tile([128, 512], f32, tag="pjb", name="pjb")
                                for ci, (k0, kl) in enumerate(CCQ):
                                    nc.tensor.matmul(
                                        ps[:mrows, :],
                                        wq_r[ci][:kl, mc0:mc0 + mrows],
                                        xq_r[ci][:kl, :],
                                        start=(ci == 0), stop=(ci == 4))
                                for ci, (k0, kl) in enumerate(CCQ):
                                    nc.tensor.matmul(
                                        pss[:mrows, :],
                                        wqs_r[ci][:kl, mc0:mc0 + mrows],
                                        xq_r[ci][:kl, :],
                                        start=(ci == 0), stop=(ci == 4))
                                dsts = ([qth[2 * m], qth[2 * m + 1]] if m < 4
                                        else [qth[8]])
                                rope(ps, pss, mrows, 512 * nn_, 512, dsts)

                            if nn_ == 1 and "attn" not in ablate:
                                # slots 3,2 only need Q window 1 -- emit them now
                                # so their PE/ACT work overlaps Q window 0
                                attn_slot(3)
                                attn_slot(2)

                # ---------- remaining attention + out-proj ---------------------
                with tc.tile_pool(name="psR", bufs=2, space="PSUM") as psR:
                    def oproj(nn_):
                        for m, (mc0, mrows) in enumerate(MM):
                            ps = psR.tile([128, 512], f32, tag="pjr", name="pjr")
                            for p, (pc0, pl) in enumerate(MM):
                                nc.tensor.matmul(
                                    ps[:mrows, :],
                                    wo_r[p][:pl, mc0:mc0 + mrows],
                                    ypr[p][:pl, 512 * nn_:512 * (nn_ + 1)],
                                    start=(p == 0), stop=(p == 4))
                            ost = pw.tile([128, 512], f32, tag="ostage", name="ostage")
                            nc.vector.tensor_copy(ost[:mrows, :], ps[:mrows, :])
                            nc.sync.dma_start(
                                yT[mc0:mc0 + mrows, 512 * nn_:512 * (nn_ + 1)],
                                ost[:mrows, :])

                    if "oproj" not in ablate:
                        oproj(1)
                    if "attn" not in ablate:
                        attn_slot(1)
                        attn_slot(0)
                    if "oproj" not in ablate:
                        oproj(0)

    nc.compile()
    return nc


def _get_program():
    global _PROG
    if _PROG is None:
        _PROG = _build_program()
    return _PROG


def _neox_perm(nheads, swap=False):
    p = []
    for h in range(nheads):
        ev = [64 * h + 2 * j for j in range(32)]
        od = [64 * h + 2 * j + 1 for j in range(32)]
        p += (od + ev) if swap else (ev + od)
    return np.array(p)


_CONSTS = None


def _static_consts():
    """Input-independent per-core constants (tables, masks, key orders)."""
    global _CONSTS
    if _CONSTS is not None:
        return _CONSTS
    invf = THETA ** (-np.arange(32, dtype=np.float64) / 32)

    def tables(pos):
        ang = pos[None, :] * invf[:, None]
        cos, sin = np.cos(ang), np.sin(ang)
        c2 = np.tile(cos, (4, 1)).astype(np.float32)
        s2 = np.tile(np.vstack([-sin, sin]), (2, 1)).astype(np.float32)
        return c2, s2

    per_j = []
    for j in range(2):
        keypos = np.concatenate(
            [np.arange(QB * q, QB * (q + 1)) for q in KEYORDER[j]])
        qsel = keypos[:TQ]          # queries = first 1024 permuted keys
        c2k, s2k = tables(keypos.astype(np.float64))
        masks = np.zeros((16 * 128, QB), np.float32)
        for s in range(4):
            seq = _slot_seq(s)
            qpos = keypos[QB * s:QB * (s + 1)]
            for k in range(4):
                c = seq[-4 + k]
                kpos = keypos[128 * c:128 * (c + 1)]
                masks[(4 * s + k) * 128:(4 * s + k + 1) * 128] = (
                    kpos[:, None] <= qpos[None, :]).astype(np.float32)
        per_j.append((keypos, qsel, c2k, s2k,
                      masks.astype(ml_dtypes.bfloat16)))
    _CONSTS = per_j
    return _CONSTS


def _host_prep(x, Wq, Wk, Wv, Wo):
    wqT = _rne12(Wq[_neox_perm(H)].T)
    wqsT = _rne12(Wq[_neox_perm(H, swap=True)].T)
    wkT = _rne12(Wk[_neox_perm(HKV)].T)
    wksT = _rne12(Wk[_neox_perm(HKV, swap=True)].T)
    woT = _rne12(Wo.T)
    wvT = np.zeros((577, 260), np.float32)
    for g in range(HKV):
        wvT[:C, 65 * g:65 * g + 64] = Wv[64 * g:64 * g + 64].T
        wvT[576, 65 * g + 64] = 1.0
    wvT = _rne12(wvT)

    per_j = _static_consts()
    x = _rne12(x)
    ones = np.ones((1, T), np.float32)
    in_maps = []
    core_meta = []
    for b in range(B):
        xbT = x[b].T
        for j in range(2):
            keypos, qsel, c2k, s2k, masks = per_j[j]
            xkT = np.vstack([xbT[:, keypos], ones])
            in_maps.append({
                "xkT": xkT,
                "wqT": wqT, "wqsT": wqsT, "wkT": wkT, "wksT": wksT,
                "wvT": wvT, "woT": woT,
                "c2k": c2k, "s2k": s2k,
                "masks": masks,
            })
            core_meta.append((b, qsel))
    return in_maps, core_meta


def kernel(x, Wq, Wk, Wv, Wo):
    x = np.asarray(x, np.float32)
    Wq = np.asarray(Wq, np.float32)
    Wk = np.asarray(Wk, np.float32)
    Wv = np.asarray(Wv, np.float32)
    Wo = np.asarray(Wo, np.float32)

    from concourse.bass_utils import run_bass_kernel_spmd

    nc = _get_program()
    in_maps, core_meta = _host_prep(x, Wq, Wk, Wv, Wo)
    res = run_bass_kernel_spmd(nc, in_maps, list(range(8)))

    out = np.empty((B, T, C), np.float32)
    for core, (b, qsel) in enumerate(core_meta):
        out[b, qsel, :] = res.results[core]["yT"].T
    return out



# revision 4
# speedup vs baseline: 1.2598x; 1.2598x over previous
"""Trainium2 Bass kernel for CausalSelfAttention (RoPE + GQA), 8-core SPMD.

Sharding: 8 cores = 4 batches x 2 query-halves (as v1). Keys PERMUTED per
core so slot s consumes the static key-chunk range [2s, 2s+PAD_s); the first
1024 permuted keys ARE the core's queries, so Q projection reuses the same
x input and RoPE tables.

v2 changes vs v1 (236us):
  - All projections in bf16 (PE cost is free-dim rows only; bf16 allows
    free<256 at full rate and halves DMA). RoPE still via double projection
    (normal + pair-swapped weights), but packed into single weight matrices
    (Q+Qs = 1152 = 9x128 cols, K+Ks = 384 = 3x128) so no 64-row matmuls.
  - PV transposed: out[q,65] = P_chunk^T @ V[keys,65] with bf16 V moving
    (free 65 vs 256 -> PV PE cost halved); ones-column gives the softmax
    denominator per q-PARTITION, so the divide is a per-partition scalar op
    (gpsimd normalize_recip) instead of reciprocal+partition_broadcast+mul.
  - y [q, feat] transposed back for the output projection with PE bf16
    transposes via an identity matrix (cheap: 128 rows each).
  - exp -> bf16 P; mask multiply all-bf16 on DVE (2x mode).
  - Projection/transpose/oproj work is emitted through a filler queue
    interleaved between attention heads so PE fills the gaps of the
    ACT(exp)-paced attention stream.
"""
import sys

sys.path.insert(0, "/opt/trn_rl_repo")

import numpy as np
import ml_dtypes

B, T, C = 4, 2048, 576
H, HKV, D = 9, 3, 64
THETA = 10000.0
QB = 256                      # query block
TQ = 1024                     # queries per core
SLOT_PAD = [16, 12, 8, 4]     # padded key-chunk counts per slot
QBLOCKS = [[7, 5, 2, 0], [6, 4, 3, 1]]   # q-256-block ids per half j
KEYORDER = [[7, 5, 2, 0, 1, 3, 4, 6], [6, 4, 3, 1, 0, 2, 5, 7]]
CCX = [(0, 128), (128, 128), (256, 128), (384, 128), (512, 65)]   # 577 rows incl ones
CCQ = [(0, 128), (128, 128), (256, 128), (384, 128), (512, 64)]   # 576-row chunks
MM = [(0, 128), (128, 128), (256, 128), (384, 128), (512, 64)]    # 576 out chunks


def _slot_seq(s):
    """Key-chunk emission order for slot s: fulls, then the two diag chunks."""
    return list(range(2 * s + 2, 2 * s + SLOT_PAD[s])) + [2 * s, 2 * s + 1]


_PROG = None


def _build_program():
    import concourse.bacc as bacc
    import concourse.mybir as mybir
    import concourse.tile as tile

    dt = mybir.dt
    f32, bf16 = dt.float32, dt.bfloat16
    AF = mybir.ActivationFunctionType

    nc = bacc.Bacc("TRN2", target_bir_lowering=False, debug=False, num_devices=8)

    def inp(name, shape, d):
        return nc.declare_dram_parameter(name, shape, d, isOutput=False)

    xkT = inp("xkT", [577, T], bf16)
    wqqs = inp("wqqs", [C, 2 * C], bf16)
    wkks = inp("wkks", [C, 2 * HKV * D], bf16)
    wvp = inp("wvp", [577, 195], bf16)
    woT = inp("woT", [C, C], bf16)
    c2k = inp("c2k", [128, T], f32)
    s2k = inp("s2k", [128, T], f32)
    masksp = inp("masks", [16 * 128, QB], bf16)
    idenp = inp("iden", [128, 128], bf16)
    yT = nc.declare_dram_parameter("yT", [C, TQ], f32, isOutput=True)

    with tile.TileContext(nc) as tc:
        with (
            tc.tile_pool(name="const", bufs=1) as cp,
            tc.tile_pool(name="rope", bufs=2) as rp,
            tc.tile_pool(name="pwork", bufs=3) as pw,
            tc.tile_pool(name="ysb", bufs=2) as ysbp,
            tc.tile_pool(name="ost", bufs=2) as ostp,
            tc.tile_pool(name="psS", bufs=2, space="PSUM") as psS,
            tc.tile_pool(name="psY", bufs=2, space="PSUM") as psY,
        ):
            # ---------------- persistent constants ----------------
            wo_r = []
            for i, (k0, kl) in enumerate(MM):
                t = cp.tile([128, C], bf16, tag=f"wo{i}", name=f"wo{i}")
                nc.sync.dma_start(t[:kl, :], woT[k0:k0 + kl, :])
                wo_r.append(t)
            m_b = cp.tile([128, 16 * QB], bf16, tag="masks", name="masks")
            for i in range(16):
                nc.sync.dma_start(m_b[:, i * QB:(i + 1) * QB],
                                  masksp[i * 128:(i + 1) * 128, :])
            c2k_t = cp.tile([128, T], f32, tag="c2k", name="c2k")
            s2k_t = cp.tile([128, T], f32, tag="s2k", name="s2k")
            id_t = cp.tile([128, 128], bf16, tag="iden", name="iden")
            nc.sync.dma_start(c2k_t[:], c2k[:])
            nc.sync.dma_start(s2k_t[:], s2k[:])
            nc.sync.dma_start(id_t[:], idenp[:])

            kt_h = [cp.tile([64, T], bf16, tag=f"kt{g}", name=f"kt{g}")
                    for g in range(HKV)]
            qth = [cp.tile([64, TQ], bf16, tag=f"qth{h}", name=f"qth{h}")
                   for h in range(H)]
            v_t = [cp.tile([128, 195], bf16, tag=f"v{c}", name=f"v{c}")
                   for c in range(16)]
            yq = [cp.tile([128, C], bf16, tag=f"yq{q}", name=f"yq{q}")
                  for q in range(8)]
            ypr = [cp.tile([128, TQ], bf16, tag=f"ypr{p}", name=f"ypr{p}")
                   for p in range(5)]

            # ---------------- attention ----------------
            fillers = []

            def pump(k=1):
                for _ in range(k):
                    if fillers:
                        fillers.pop(0)()

            def attn_slot(s):
                seq = _slot_seq(s)
                n = len(seq)
                for h in range(H):
                    g = h // 3
                    yh = [psY.tile([128, 65], f32, tag="ypsum", name="ypsum",
                                   padded_shape=[128, 512]) for _ in range(2)]
                    for sc in range(n // 4):
                        sp = psS.tile([128, 4 * QB], f32, tag="scores",
                                      name="scores")
                        for i in range(4):
                            c = seq[4 * sc + i]
                            nc.tensor.matmul(
                                sp[:, QB * i:QB * (i + 1)],
                                kt_h[g][0:64, 128 * c:128 * (c + 1)],
                                qth[h][0:64, QB * s:QB * (s + 1)],
                                start=True, stop=True)
                        p_b = pw.tile([128, 4 * QB], bf16, tag="p", name="p")
                        nc.scalar.activation(p_b[:], sp[:], AF.Exp, scale=0.125)
                        if sc == n // 4 - 1:
                            nc.vector.tensor_mul(
                                p_b[:], p_b[:],
                                m_b[:, 1024 * s:1024 * (s + 1)])
                        for i in range(4):
                            c = seq[4 * sc + i]
                            ci = 4 * sc + i
                            for hf in range(2):
                                nc.tensor.matmul(
                                    yh[hf][:, 0:65],
                                    p_b[:, QB * i + 128 * hf:
                                        QB * i + 128 * hf + 128],
                                    v_t[c][:, 65 * g:65 * g + 65],
                                    start=(ci == 0), stop=(ci == n - 1))
                    for hf in range(2):
                        ys = ysbp.tile([128, 65], f32, tag="ysb", name="ysb")
                        nc.vector.tensor_copy(ys[:], yh[hf][:])
                        nc.gpsimd.normalize_recip(
                            yq[2 * s + hf][:, 64 * h:64 * h + 64],
                            ys[:, 0:64], ys[:, 64:65])
                    if h in (2, 5, 7):
                        pump()

            # ---------------- projections (phase 1+2) ----------------
            with (
                tc.tile_pool(name="wp", bufs=1) as wp,
                tc.tile_pool(name="psA", bufs=1, space="PSUM") as psA,
                tc.tile_pool(name="psB", bufs=1, space="PSUM") as psB,
                tc.tile_pool(name="xk", bufs=2) as xkp,
            ):
                def load_w(param, chunks, cols, tag):
                    tiles = []
                    for i, (k0, kl) in enumerate(chunks):
                        t = wp.tile([128, cols], bf16, tag=f"{tag}{i}",
                                    name=f"{tag}{i}")
                        nc.sync.dma_start(t[:kl, :], param[k0:k0 + kl, :])
                        tiles.append(t)
                    return tiles

                wq_r = load_w(wqqs, CCQ, 2 * C, "wq")
                wk_r = load_w(wkks, CCQ, 2 * HKV * D, "wk")
                wv_r = load_w(wvp, CCX, 195, "wv")

                def load_x(win):
                    xk_r = []
                    for i, (k0, kl) in enumerate(CCX):
                        t = xkp.tile([128, 512], bf16, tag=f"xk{i}",
                                     name=f"xk{i}")
                        nc.sync.dma_start(
                            t[:kl, :], xkT[k0:k0 + kl, 512 * win:512 * (win + 1)])
                        xk_r.append(t)
                    return xk_r

                def pj(pool, tag, w_r, mi, xk_r, ncols=512):
                    ps = pool.tile([128, 512], f32, tag=tag, name=tag)
                    for ci, (k0, kl) in enumerate(CCQ):
                        nc.tensor.matmul(
                            ps[:, :ncols],
                            w_r[ci][:kl, 128 * mi:128 * (mi + 1)],
                            xk_r[ci][:kl, :ncols],
                            start=(ci == 0), stop=(ci == 4))
                    return ps

                def kproj(win, xk_r):
                    # wkks cols: [K g0,g1 | K g2, Ks g0 | Ks g1, Ks g2].
                    # Swapped-side muls are written cross-base so each add's
                    # two inputs share a base partition (verifier rule).
                    c0 = 512 * win
                    ps = [pj(psA if mi % 2 == 0 else psB,
                             "pja" if mi % 2 == 0 else "pjb",
                             wk_r, mi, xk_r) for mi in range(3)]
                    t1a = rp.tile([128, 512], bf16, tag="kt1a", name="kt1a")
                    t1b = rp.tile([64, 512], bf16, tag="kt1b", name="kt1b")
                    t2b = rp.tile([64, 512], bf16, tag="kt2b", name="kt2b")
                    t2a = rp.tile([128, 512], bf16, tag="kt2a", name="kt2a")
                    nc.vector.tensor_mul(t1a[:], ps[0][:], c2k_t[:, c0:c0 + 512])
                    nc.vector.tensor_mul(t1b[:], ps[1][0:64, :],
                                         c2k_t[0:64, c0:c0 + 512])
                    # Ks g0 at ps[1] rows 64:128 -> base 0
                    nc.vector.tensor_mul(t2b[0:64, :], ps[1][64:128, :],
                                         s2k_t[64:128, c0:c0 + 512])
                    # Ks g1 at ps[2] rows 0:64 -> base 64
                    nc.vector.tensor_mul(t2a[64:128, :], ps[2][0:64, :],
                                         s2k_t[0:64, c0:c0 + 512])
                    # Ks g2 at ps[2] rows 64:128 -> base 0
                    nc.vector.tensor_mul(t2a[0:64, :], ps[2][64:128, :],
                                         s2k_t[64:128, c0:c0 + 512])
                    nc.gpsimd.tensor_add(kt_h[0][0:64, c0:c0 + 512],
                                         t1a[0:64, :], t2b[0:64, :])
                    nc.gpsimd.tensor_add(kt_h[1][0:64, c0:c0 + 512],
                                         t1a[64:128, :], t2a[64:128, :])
                    nc.gpsimd.tensor_add(kt_h[2][0:64, c0:c0 + 512],
                                         t1b[0:64, :], t2a[0:64, :])

                def vproj(win, xk_r, ti):
                    t_ = 4 * win + ti
                    pool = psA if ti % 2 == 0 else psB
                    tag = "pja" if ti % 2 == 0 else "pjb"
                    ps = pool.tile([128, 512], f32, tag=tag, name=tag)
                    for ci, (k0, kl) in enumerate(CCX):
                        nc.tensor.matmul(
                            ps[:, 0:195],
                            xk_r[ci][:kl, 128 * ti:128 * (ti + 1)],
                            wv_r[ci][:kl, :],
                            start=(ci == 0), stop=(ci == 4))
                    nc.vector.tensor_copy(v_t[t_][:], ps[:, 0:195])

                def qproj(win, xk_r, lo, hi):
                    # wqqs cols: [Q h0..h8 | Qs h0..h8]. Qs h sits at col
                    # 576+64h (opposite 64-parity to Q h), so swapped muls
                    # write cross-base to align each add's input pair.
                    c0 = 512 * win
                    t1, t2 = {}, {}
                    for mi in range(lo, hi):
                        ps = pj(psA if mi % 2 == 0 else psB,
                                "pja" if mi % 2 == 0 else "pjb",
                                wq_r, mi, xk_r)
                        if mi <= 3:
                            t = rp.tile([128, 512], bf16, tag=f"qt1_{mi}",
                                        name=f"qt1_{mi}")
                            nc.vector.tensor_mul(t[:], ps[:],
                                                 c2k_t[:, c0:c0 + 512])
                            t1[mi] = t
                        elif mi == 4:
                            ta = rp.tile([64, 512], bf16, tag="qt1_4",
                                         name="qt1_4")
                            tb = rp.tile([64, 512], bf16, tag="qt2_4",
                                         name="qt2_4")
                            nc.vector.tensor_mul(ta[:], ps[0:64, :],
                                                 c2k_t[0:64, c0:c0 + 512])
                            # Qs h0 at rows 64:128 -> base 0
                            nc.vector.tensor_mul(tb[0:64, :], ps[64:128, :],
                                                 s2k_t[64:128, c0:c0 + 512])
                            t1[4], t2[4] = ta, tb
                        else:
                            t = rp.tile([128, 512], bf16, tag=f"qt2_{mi}",
                                        name=f"qt2_{mi}")
                            # rows 0:64 hold Qs h(odd-src), cross-based
                            nc.vector.tensor_mul(t[64:128, :], ps[0:64, :],
                                                 s2k_t[0:64, c0:c0 + 512])
                            nc.vector.tensor_mul(t[0:64, :], ps[64:128, :],
                                                 s2k_t[64:128, c0:c0 + 512])
                            t2[mi] = t
                    return t1, t2

                def qrope(win, t1, t2):
                    c0 = 512 * win
                    for h in range(H):
                        bd = 64 * (h % 2)
                        a = t1[h // 2]
                        b = t2[(576 + 64 * h) // 128]
                        nc.gpsimd.tensor_add(
                            qth[h][0:64, c0:c0 + 512],
                            a[bd:bd + 64, :], b[bd:bd + 64, :])

                def full_win(win, with_q):
                    xk_r = load_x(win)
                    kproj(win, xk_r)
                    for ti in range(4):
                        vproj(win, xk_r, ti)
                    if with_q:
                        t1, t2 = qproj(win, xk_r, 0, 9)
                        qrope(win, t1, t2)

                # windows 1, 2 serial up front (slot 3+2 deps), then slots
                # 3,2,1 with windows 0,3 interleaved via the filler queue.
                full_win(1, True)
                full_win(2, False)

                def win_filler(win, with_q):
                    st = {}

                    def f_load():
                        st["xk"] = load_x(win)

                    def f_k():
                        kproj(win, st["xk"])

                    def f_v(ti):
                        return lambda: vproj(win, st["xk"], ti)

                    def f_q(lo, hi):
                        def g():
                            t1, t2 = qproj(win, st["xk"], lo, hi)
                            st.setdefault("t1", {}).update(t1)
                            st.setdefault("t2", {}).update(t2)
                        return g

                    def f_qrope():
                        qrope(win, st["t1"], st["t2"])

                    units = [f_load, f_k, f_v(0), f_v(1), f_v(2), f_v(3)]
                    if with_q:
                        units += [f_q(0, 3), f_q(3, 6), f_q(6, 9), f_qrope]
                    return units

                fillers.extend(win_filler(0, True))
                fillers.extend(win_filler(3, False))

                attn_slot(3)
                attn_slot(2)
                while fillers:
                    pump()
                attn_slot(1)

            # ---------------- transposes, out-proj, last slot ----------------
            with (
                tc.tile_pool(name="psT", bufs=1, space="PSUM") as psT,
                tc.tile_pool(name="psR", bufs=1, space="PSUM") as psR,
            ):
                def transp(qc):
                    def g():
                        for m, (mc0, mrows) in enumerate(MM):
                            pt = psT.tile([128, 128], bf16, tag="pt", name="pt",
                                          padded_shape=[128, 1024])
                            nc.tensor.matmul(pt[:mrows, :],
                                             yq[qc][:, mc0:mc0 + mrows],
                                             id_t[:], start=True, stop=True,
                                             is_transpose=True)
                            nc.vector.tensor_copy(
                                ypr[m][0:mrows, 128 * qc:128 * (qc + 1)],
                                pt[:mrows, :])
                    return g

                def oproj_m(nn, m):
                    def g():
                        mc0, mrows = MM[m]
                        ps = psR.tile([128, 512], f32, tag="pjr", name="pjr")
                        for p, (pc0, pl) in enumerate(MM):
                            nc.tensor.matmul(
                                ps[:mrows, :],
                                wo_r[p][:pl, mc0:mc0 + mrows],
                                ypr[p][:pl, 512 * nn:512 * (nn + 1)],
                                start=(p == 0), stop=(p == 4))
                        ost = ostp.tile([128, 512], f32, tag="ostage",
                                        name="ostage")
                        nc.vector.tensor_copy(ost[:mrows, :], ps[:mrows, :])
                        nc.sync.dma_start(
                            yT[mc0:mc0 + mrows, 512 * nn:512 * (nn + 1)],
                            ost[:mrows, :])
                    return g

                fillers.extend([transp(6), transp(7), transp(4), transp(5),
                                transp(2), transp(3)])
                fillers.extend([oproj_m(1, m) for m in range(5)])
                attn_slot(0)
                while fillers:
                    pump()
                transp(0)()
                transp(1)()
                for m in range(5):
                    oproj_m(0, m)()

    nc.compile()
    return nc


def _get_program():
    global _PROG
    if _PROG is None:
        _PROG = _build_program()
    return _PROG


def _neox_perm(nheads, swap=False):
    p = []
    for h in range(nheads):
        ev = [64 * h + 2 * j for j in range(32)]
        od = [64 * h + 2 * j + 1 for j in range(32)]
        p += (od + ev) if swap else (ev + od)
    return np.array(p)


_CONSTS = None


def _static_consts():
    """Input-independent per-core constants (tables, masks, key orders)."""
    global _CONSTS
    if _CONSTS is not None:
        return _CONSTS
    invf = THETA ** (-np.arange(32, dtype=np.float64) / 32)

    def tables(pos):
        ang = pos[None, :] * invf[:, None]
        cos, sin = np.cos(ang), np.sin(ang)
        c2 = np.tile(cos, (4, 1)).astype(np.float32)
        s2 = np.tile(np.vstack([-sin, sin]), (2, 1)).astype(np.float32)
        return c2, s2

    per_j = []
    for j in range(2):
        keypos = np.concatenate(
            [np.arange(QB * q, QB * (q + 1)) for q in KEYORDER[j]])
        qsel = keypos[:TQ]          # queries = first 1024 permuted keys
        c2k, s2k = tables(keypos.astype(np.float64))
        masks = np.zeros((16 * 128, QB), np.float32)
        for s in range(4):
            seq = _slot_seq(s)
            qpos = keypos[QB * s:QB * (s + 1)]
            for k in range(4):
                c = seq[-4 + k]
                kpos = keypos[128 * c:128 * (c + 1)]
                masks[(4 * s + k) * 128:(4 * s + k + 1) * 128] = (
                    kpos[:, None] <= qpos[None, :]).astype(np.float32)
        per_j.append((keypos, qsel, c2k, s2k,
                      masks.astype(ml_dtypes.bfloat16)))
    _CONSTS = per_j
    return _CONSTS


def _host_prep(x, Wq, Wk, Wv, Wo):
    bf = ml_dtypes.bfloat16
    wqqs = np.hstack([Wq[_neox_perm(H)].T,
                      Wq[_neox_perm(H, swap=True)].T]).astype(bf)
    wkks = np.hstack([Wk[_neox_perm(HKV)].T,
                      Wk[_neox_perm(HKV, swap=True)].T]).astype(bf)
    woT = Wo.T.astype(bf)
    wvp = np.zeros((577, 195), np.float32)
    for g in range(HKV):
        wvp[:C, 65 * g:65 * g + 64] = Wv[64 * g:64 * g + 64].T
        wvp[576, 65 * g + 64] = 1.0
    wvp = wvp.astype(bf)
    iden = np.eye(128, dtype=np.float32).astype(bf)

    per_j = _static_consts()
    ones = np.ones((1, T), np.float32)
    in_maps = []
    core_meta = []
    for b in range(B):
        xbT = x[b].T
        for j in range(2):
            keypos, qsel, c2k, s2k, masks = per_j[j]
            xkT = np.vstack([xbT[:, keypos], ones]).astype(bf)
            in_maps.append({
                "xkT": xkT,
                "wqqs": wqqs, "wkks": wkks, "wvp": wvp, "woT": woT,
                "c2k": c2k, "s2k": s2k,
                "masks": masks, "iden": iden,
            })
            core_meta.append((b, qsel))
    return in_maps, core_meta


def kernel(x, Wq, Wk, Wv, Wo):
    x = np.asarray(x, np.float32)
    Wq = np.asarray(Wq, np.float32)
    Wk = np.asarray(Wk, np.float32)
    Wv = np.asarray(Wv, np.float32)
    Wo = np.asarray(Wo, np.float32)

    from concourse.bass_utils import run_bass_kernel_spmd

    nc = _get_program()
    in_maps, core_meta = _host_prep(x, Wq, Wk, Wv, Wo)
    res = run_bass_kernel_spmd(nc, in_maps, list(range(8)))

    out = np.empty((B, T, C), np.float32)
    for core, (b, qsel) in enumerate(core_meta):
        out[b, qsel, :] = res.results[core]["yT"].T
    return out


# revision 6
# speedup vs baseline: 1.4355x; 1.1395x over previous
"""Trainium2 Bass kernel for CausalSelfAttention (RoPE + GQA), 8-core SPMD.

Sharding: 8 cores = 4 batches x 2 query-halves (as v1). Keys PERMUTED per
core so slot s consumes the static key-chunk range [2s, 2s+PAD_s); the first
1024 permuted keys ARE the core's queries, so Q projection reuses the same
x input and RoPE tables.

v2 changes vs v1 (236us):
  - All projections in bf16 (PE cost is free-dim rows only; bf16 allows
    free<256 at full rate and halves DMA). RoPE still via double projection
    (normal + pair-swapped weights), but packed into single weight matrices
    (Q+Qs = 1152 = 9x128 cols, K+Ks = 384 = 3x128) so no 64-row matmuls.
  - PV transposed: out[q,65] = P_chunk^T @ V[keys,65] with bf16 V moving
    (free 65 vs 256 -> PV PE cost halved); ones-column gives the softmax
    denominator per q-PARTITION, so the divide is a per-partition scalar op
    (gpsimd normalize_recip) instead of reciprocal+partition_broadcast+mul.
  - y [q, feat] transposed back for the output projection with PE bf16
    transposes via an identity matrix (cheap: 128 rows each).
  - exp -> bf16 P; mask multiply all-bf16 on DVE (2x mode).
  - Projection/transpose/oproj work is emitted through a filler queue
    interleaved between attention heads so PE fills the gaps of the
    ACT(exp)-paced attention stream.
"""
import sys

sys.path.insert(0, "/opt/trn_rl_repo")

import numpy as np
import ml_dtypes

B, T, C = 4, 2048, 576
H, HKV, D = 9, 3, 64
THETA = 10000.0
QB = 256                      # query block
TQ = 1024                     # queries per core
SLOT_PAD = [16, 12, 8, 4]     # padded key-chunk counts per slot
QBLOCKS = [[7, 5, 2, 0], [6, 4, 3, 1]]   # q-256-block ids per half j
KEYORDER = [[7, 5, 2, 0, 1, 3, 4, 6], [6, 4, 3, 1, 0, 2, 5, 7]]
CCX = [(0, 128), (128, 128), (256, 128), (384, 128), (512, 65)]   # 577 rows incl ones
CCQ = [(0, 128), (128, 128), (256, 128), (384, 128), (512, 64)]   # 576-row chunks
MM = [(0, 128), (128, 128), (256, 128), (384, 128), (512, 64)]    # 576 out chunks


def _slot_seq(s):
    """Key-chunk emission order for slot s: fulls, then the two diag chunks."""
    return list(range(2 * s + 2, 2 * s + SLOT_PAD[s])) + [2 * s, 2 * s + 1]


_PROG = None


def _build_program():
    import concourse.bacc as bacc
    import concourse.mybir as mybir
    import concourse.tile as tile

    dt = mybir.dt
    f32, bf16 = dt.float32, dt.bfloat16
    AF = mybir.ActivationFunctionType

    nc = bacc.Bacc("TRN2", target_bir_lowering=False, debug=False, num_devices=8)

    def inp(name, shape, d):
        return nc.declare_dram_parameter(name, shape, d, isOutput=False)

    xkT = inp("xkT", [577, T], bf16)
    wqqs = inp("wqqs", [C, 2 * C], bf16)
    wkks = inp("wkks", [C, 2 * HKV * D], bf16)
    wvp = inp("wvp", [577, 195], bf16)
    woT = inp("woT", [C, C], bf16)
    c2k = inp("c2k", [128, T], bf16)
    s2k = inp("s2k", [128, T], bf16)
    masksp = inp("masks", [128, 16 * QB], bf16)
    idenp = inp("iden", [128, 128], bf16)
    yT = nc.declare_dram_parameter("yT", [C, TQ], f32, isOutput=True)

    with tile.TileContext(nc) as tc:
        with (
            tc.tile_pool(name="const", bufs=1) as cp,
            tc.tile_pool(name="rope", bufs=2) as rp,
            tc.tile_pool(name="pwork", bufs=3) as pw,
            tc.tile_pool(name="ysb", bufs=2) as ysbp,
            tc.tile_pool(name="ost", bufs=2) as ostp,
            tc.tile_pool(name="psS", bufs=2, space="PSUM") as psS,
            tc.tile_pool(name="psY", bufs=2, space="PSUM") as psY,
        ):
            # ---------------- persistent constants ----------------
            # (DMA emission for most constants is deferred into the
            # projection phase so the first window's x/wk loads go first.)
            wo_r = [cp.tile([128, C], bf16, tag=f"wo{i}", name=f"wo{i}")
                    for i in range(5)]
            m_b = cp.tile([128, 16 * QB], bf16, tag="masks", name="masks")
            c2k_t = cp.tile([128, T], bf16, tag="c2k", name="c2k")
            s2k_t = cp.tile([128, T], bf16, tag="s2k", name="s2k")
            id_t = cp.tile([128, 128], bf16, tag="iden", name="iden")

            kt_h = [cp.tile([64, T], bf16, tag=f"kt{g}", name=f"kt{g}")
                    for g in range(HKV)]
            qth = [cp.tile([64, TQ], bf16, tag=f"qth{h}", name=f"qth{h}")
                   for h in range(H)]
            v_t = [cp.tile([128, 195], bf16, tag=f"v{c}", name=f"v{c}")
                   for c in range(16)]
            yq = [cp.tile([128, C], bf16, tag=f"yq{q}", name=f"yq{q}")
                  for q in range(8)]
            ypr = [cp.tile([128, TQ], bf16, tag=f"ypr{p}", name=f"ypr{p}")
                   for p in range(5)]

            # ---------------- attention ----------------
            fillers = []

            def pump(k=1):
                for _ in range(k):
                    if fillers:
                        fillers.pop(0)()

            def attn_slot(s):
                seq = _slot_seq(s)
                n = len(seq)
                for h in range(H):
                    g = h // 3
                    yh = [psY.tile([128, 65], f32, tag="ypsum", name="ypsum",
                                   padded_shape=[128, 512]) for _ in range(2)]
                    for sc in range(n // 4):
                        sp = psS.tile([128, 4 * QB], f32, tag="scores",
                                      name="scores")
                        for i in range(4):
                            c = seq[4 * sc + i]
                            nc.tensor.matmul(
                                sp[:, QB * i:QB * (i + 1)],
                                kt_h[g][0:64, 128 * c:128 * (c + 1)],
                                qth[h][0:64, QB * s:QB * (s + 1)],
                                start=True, stop=True)
                        p_b = pw.tile([128, 4 * QB], bf16, tag="p", name="p")
                        nc.scalar.activation(p_b[:], sp[:], AF.Exp, scale=0.125)
                        if sc == n // 4 - 1:
                            nc.vector.tensor_mul(
                                p_b[:], p_b[:],
                                m_b[:, 1024 * s:1024 * (s + 1)])
                        for i in range(4):
                            c = seq[4 * sc + i]
                            ci = 4 * sc + i
                            for hf in range(2):
                                nc.tensor.matmul(
                                    yh[hf][:, 0:65],
                                    p_b[:, QB * i + 128 * hf:
                                        QB * i + 128 * hf + 128],
                                    v_t[c][:, 65 * g:65 * g + 65],
                                    start=(ci == 0), stop=(ci == n - 1))
                    for hf in range(2):
                        ys = ysbp.tile([128, 65], f32, tag="ysb", name="ysb")
                        nc.vector.tensor_copy(ys[:], yh[hf][:])
                        nc.gpsimd.normalize_recip(
                            yq[2 * s + hf][:, 64 * h:64 * h + 64],
                            ys[:, 0:64], ys[:, 64:65])
                    pump()

            # ---------------- projections (phase 1+2) ----------------
            with (
                tc.tile_pool(name="wp", bufs=1) as wp,
                tc.tile_pool(name="psA", bufs=1, space="PSUM") as psA,
                tc.tile_pool(name="psB", bufs=1, space="PSUM") as psB,
                tc.tile_pool(name="xk", bufs=2) as xkp,
            ):
                def load_w(param, chunks, cols, tag):
                    tiles = []
                    for i, (k0, kl) in enumerate(chunks):
                        t = wp.tile([128, cols], bf16, tag=f"{tag}{i}",
                                    name=f"{tag}{i}")
                        nc.sync.dma_start(t[:kl, :], param[k0:k0 + kl, :])
                        tiles.append(t)
                    return tiles

                def load_x(win):
                    xk_r = []
                    for i, (k0, kl) in enumerate(CCX):
                        t = xkp.tile([128, 512], bf16, tag=f"xk{i}",
                                     name=f"xk{i}")
                        nc.sync.dma_start(
                            t[:kl, :], xkT[k0:k0 + kl, 512 * win:512 * (win + 1)])
                        xk_r.append(t)
                    return xk_r

                pjc = [0]

                def pj(w_r, mi, xk_r):
                    # alternate psA/psB per emitted group so bufs=1 reuse is
                    # hidden behind the intervening group's matmuls
                    pool, tag = ((psA, "pja") if pjc[0] % 2 == 0
                                 else (psB, "pjb"))
                    pjc[0] += 1
                    ps = pool.tile([128, 512], f32, tag=tag, name=tag)
                    for ci, (k0, kl) in enumerate(CCQ):
                        nc.tensor.matmul(
                            ps[:, :],
                            w_r[ci][:kl, 128 * mi:128 * (mi + 1)],
                            xk_r[ci][:kl, :],
                            start=(ci == 0), stop=(ci == 4))
                    return ps

                def kproj_a(win, xk_r, st):
                    # wkks cols: [K g0,g1 | K g2, Ks g0 | Ks g1, Ks g2].
                    # Swapped-side muls are written cross-base so each add's
                    # two inputs share a base partition (verifier rule).
                    c0 = 512 * win
                    ps0 = pj(wk_r, 0, xk_r)
                    t1a = rp.tile([128, 512], bf16, tag="kt1a", name="kt1a")
                    nc.vector.tensor_mul(t1a[:], ps0[:], c2k_t[:, c0:c0 + 512])
                    ps1 = pj(wk_r, 1, xk_r)
                    t1b = rp.tile([64, 512], bf16, tag="kt1b", name="kt1b")
                    t2b = rp.tile([64, 512], bf16, tag="kt2b", name="kt2b")
                    nc.vector.tensor_mul(t1b[:], ps1[0:64, :],
                                         c2k_t[0:64, c0:c0 + 512])
                    nc.vector.tensor_mul(t2b[0:64, :], ps1[64:128, :],
                                         s2k_t[64:128, c0:c0 + 512])
                    st.update(t1a=t1a, t1b=t1b, t2b=t2b)

                def kproj_b(win, xk_r, st):
                    c0 = 512 * win
                    ps2 = pj(wk_r, 2, xk_r)
                    t2a = rp.tile([128, 512], bf16, tag="kt2a", name="kt2a")
                    nc.vector.tensor_mul(t2a[64:128, :], ps2[0:64, :],
                                         s2k_t[0:64, c0:c0 + 512])
                    nc.vector.tensor_mul(t2a[0:64, :], ps2[64:128, :],
                                         s2k_t[64:128, c0:c0 + 512])
                    t1a, t1b, t2b = st["t1a"], st["t1b"], st["t2b"]
                    nc.gpsimd.tensor_add(kt_h[0][0:64, c0:c0 + 512],
                                         t1a[0:64, :], t2b[0:64, :])
                    nc.gpsimd.tensor_add(kt_h[1][0:64, c0:c0 + 512],
                                         t1a[64:128, :], t2a[64:128, :])
                    nc.gpsimd.tensor_add(kt_h[2][0:64, c0:c0 + 512],
                                         t1b[0:64, :], t2a[0:64, :])

                def vproj(win, xk_r, ti):
                    t_ = 4 * win + ti
                    pool, tag = ((psA, "pja") if pjc[0] % 2 == 0
                                 else (psB, "pjb"))
                    pjc[0] += 1
                    ps = pool.tile([128, 512], f32, tag=tag, name=tag)
                    for ci, (k0, kl) in enumerate(CCX):
                        nc.tensor.matmul(
                            ps[:, 0:195],
                            xk_r[ci][:kl, 128 * ti:128 * (ti + 1)],
                            wv_r[ci][:kl, :],
                            start=(ci == 0), stop=(ci == 4))
                    nc.vector.tensor_copy(v_t[t_][:], ps[:, 0:195])

                def qproj(win, xk_r, lo, hi):
                    # wqqs cols: [Q h0..h8 | Qs h0..h8]. Qs h sits at col
                    # 576+64h (opposite 64-parity to Q h), so swapped muls
                    # write cross-base to align each add's input pair.
                    c0 = 512 * win
                    t1, t2 = {}, {}
                    for mi in range(lo, hi):
                        ps = pj(wq_r, mi, xk_r)
                        if mi <= 3:
                            t = rp.tile([128, 512], bf16, tag=f"qt1_{mi}",
                                        name=f"qt1_{mi}")
                            nc.vector.tensor_mul(t[:], ps[:],
                                                 c2k_t[:, c0:c0 + 512])
                            t1[mi] = t
                        elif mi == 4:
                            ta = rp.tile([64, 512], bf16, tag="qt1_4",
                                         name="qt1_4")
                            tb = rp.tile([64, 512], bf16, tag="qt2_4",
                                         name="qt2_4")
                            nc.vector.tensor_mul(ta[:], ps[0:64, :],
                                                 c2k_t[0:64, c0:c0 + 512])
                            # Qs h0 at rows 64:128 -> base 0
                            nc.vector.tensor_mul(tb[0:64, :], ps[64:128, :],
                                                 s2k_t[64:128, c0:c0 + 512])
                            t1[4], t2[4] = ta, tb
                        else:
                            t = rp.tile([128, 512], bf16, tag=f"qt2_{mi}",
                                        name=f"qt2_{mi}")
                            # rows 0:64 hold Qs h(odd-src), cross-based
                            nc.vector.tensor_mul(t[64:128, :], ps[0:64, :],
                                                 s2k_t[0:64, c0:c0 + 512])
                            nc.vector.tensor_mul(t[0:64, :], ps[64:128, :],
                                                 s2k_t[64:128, c0:c0 + 512])
                            t2[mi] = t
                    return t1, t2

                def qrope(win, t1, t2):
                    c0 = 512 * win
                    for h in range(H):
                        bd = 64 * (h % 2)
                        a = t1[h // 2]
                        b = t2[(576 + 64 * h) // 128]
                        nc.gpsimd.tensor_add(
                            qth[h][0:64, c0:c0 + 512],
                            a[bd:bd + 64, :], b[bd:bd + 64, :])

                def full_win(win, with_q, xk_r=None):
                    if xk_r is None:
                        xk_r = load_x(win)
                    st = {}
                    kproj_a(win, xk_r, st)
                    vproj(win, xk_r, 0)
                    kproj_b(win, xk_r, st)
                    for ti in range(1, 4):
                        vproj(win, xk_r, ti)
                    if with_q:
                        t1, t2 = qproj(win, xk_r, 0, 9)
                        qrope(win, t1, t2)

                # DMA order: first window's x + wk + tables first, so the
                # first K matmul starts a few us in; bulky/late-needed
                # constants (masks, wo) go after window 1's compute.
                xk1 = load_x(1)
                wk_r = load_w(wkks, CCQ, 2 * HKV * D, "wk")
                nc.sync.dma_start(c2k_t[:], c2k[:])
                nc.sync.dma_start(s2k_t[:], s2k[:])
                wv_r = load_w(wvp, CCX, 195, "wv")
                wq_r = load_w(wqqs, CCQ, 2 * C, "wq")

                # windows 1, 2 serial up front (slot 3+2 deps), then slots
                # 3,2,1 with windows 0,3 interleaved via the filler queue.
                full_win(1, True, xk1)
                nc.sync.dma_start(m_b[:], masksp[:])
                nc.sync.dma_start(id_t[:], idenp[:])
                for i, (k0, kl) in enumerate(MM):
                    nc.sync.dma_start(wo_r[i][:kl, :], woT[k0:k0 + kl, :])
                full_win(2, False)

                def win_filler(win, with_q):
                    st = {}

                    def f_load():
                        st["xk"] = load_x(win)

                    def f_ka():
                        kproj_a(win, st["xk"], st)

                    def f_kb():
                        kproj_b(win, st["xk"], st)

                    def f_v(ti):
                        return lambda: vproj(win, st["xk"], ti)

                    def f_q(lo, hi):
                        def g():
                            t1, t2 = qproj(win, st["xk"], lo, hi)
                            st.setdefault("t1", {}).update(t1)
                            st.setdefault("t2", {}).update(t2)
                        return g

                    def f_qrope():
                        qrope(win, st["t1"], st["t2"])

                    units = [f_load, f_ka, f_v(0), f_kb,
                             f_v(1), f_v(2), f_v(3)]
                    if with_q:
                        units += [f_q(0, 3), f_q(3, 6), f_q(6, 9), f_qrope]
                    return units

                fillers.extend(win_filler(0, True))
                fillers.extend(win_filler(3, False))

                attn_slot(3)
                attn_slot(2)
                while fillers:
                    pump()
                attn_slot(1)

            # ---------------- transposes, out-proj, last slot ----------------
            with (
                tc.tile_pool(name="psT", bufs=1, space="PSUM") as psT,
                tc.tile_pool(name="psR", bufs=1, space="PSUM") as psR,
            ):
                def transp(qc):
                    def g():
                        for m, (mc0, mrows) in enumerate(MM):
                            pt = psT.tile([128, 128], bf16, tag="pt", name="pt",
                                          padded_shape=[128, 1024])
                            nc.tensor.matmul(pt[:mrows, :],
                                             yq[qc][:, mc0:mc0 + mrows],
                                             id_t[:], start=True, stop=True,
                                             is_transpose=True)
                            nc.vector.tensor_copy(
                                ypr[m][0:mrows, 128 * qc:128 * (qc + 1)],
                                pt[:mrows, :])
                    return g

                def oproj_m(qq, m):
                    # qq = 256-col q-quarter; quarter 2s/2s+1 ready as soon
                    # as slot s is transposed, so most of oproj hides inside
                    # the remaining attention stream.
                    def g():
                        mc0, mrows = MM[m]
                        ps = psR.tile([128, 256], f32, tag="pjr", name="pjr",
                                      padded_shape=[128, 512])
                        for p, (pc0, pl) in enumerate(MM):
                            nc.tensor.matmul(
                                ps[:mrows, :],
                                wo_r[p][:pl, mc0:mc0 + mrows],
                                ypr[p][:pl, 256 * qq:256 * (qq + 1)],
                                start=(p == 0), stop=(p == 4))
                        ost = ostp.tile([128, 256], f32, tag="ostage",
                                        name="ostage")
                        nc.vector.tensor_copy(ost[:mrows, :], ps[:mrows, :])
                        nc.sync.dma_start(
                            yT[mc0:mc0 + mrows, 256 * qq:256 * (qq + 1)],
                            ost[:mrows, :])
                    return g

                fillers.extend([transp(6), transp(7)])
                fillers.extend([oproj_m(3, m) for m in range(5)])
                fillers.extend([transp(4), transp(5)])
                fillers.extend([oproj_m(2, m) for m in range(5)])
                fillers.extend([transp(2), transp(3)])
                fillers.extend([oproj_m(1, m) for m in range(5)])
                attn_slot(0)
                while fillers:
                    pump()
                transp(0)()
                transp(1)()
                for m in range(5):
                    oproj_m(0, m)()

    nc.compile()
    return nc


def _get_program():
    global _PROG
    if _PROG is None:
        _PROG = _build_program()
    return _PROG


def _neox_perm(nheads, swap=False):
    p = []
    for h in range(nheads):
        ev = [64 * h + 2 * j for j in range(32)]
        od = [64 * h + 2 * j + 1 for j in range(32)]
        p += (od + ev) if swap else (ev + od)
    return np.array(p)


_CONSTS = None


def _static_consts():
    """Input-independent per-core constants (tables, masks, key orders)."""
    global _CONSTS
    if _CONSTS is not None:
        return _CONSTS
    invf = THETA ** (-np.arange(32, dtype=np.float64) / 32)

    def tables(pos):
        ang = pos[None, :] * invf[:, None]
        cos, sin = np.cos(ang), np.sin(ang)
        c2 = np.tile(cos, (4, 1)).astype(np.float32)
        s2 = np.tile(np.vstack([-sin, sin]), (2, 1)).astype(np.float32)
        return c2, s2

    per_j = []
    for j in range(2):
        keypos = np.concatenate(
            [np.arange(QB * q, QB * (q + 1)) for q in KEYORDER[j]])
        qsel = keypos[:TQ]          # queries = first 1024 permuted keys
        c2k, s2k = tables(keypos.astype(np.float64))
        masks = np.zeros((16 * 128, QB), np.float32)
        for s in range(4):
            seq = _slot_seq(s)
            qpos = keypos[QB * s:QB * (s + 1)]
            for k in range(4):
                c = seq[-4 + k]
                kpos = keypos[128 * c:128 * (c + 1)]
                masks[(4 * s + k) * 128:(4 * s + k + 1) * 128] = (
                    kpos[:, None] <= qpos[None, :]).astype(np.float32)
        # device layout: [128, 16*QB] (16 chunk-masks side by side)
        masks2 = masks.reshape(16, 128, QB).transpose(1, 0, 2).reshape(128, 16 * QB)
        per_j.append((keypos, qsel,
                      c2k.astype(ml_dtypes.bfloat16),
                      s2k.astype(ml_dtypes.bfloat16),
                      masks2.astype(ml_dtypes.bfloat16)))
    _CONSTS = per_j
    return _CONSTS


def _host_prep(x, Wq, Wk, Wv, Wo):
    bf = ml_dtypes.bfloat16
    wqqs = np.hstack([Wq[_neox_perm(H)].T,
                      Wq[_neox_perm(H, swap=True)].T]).astype(bf)
    wkks = np.hstack([Wk[_neox_perm(HKV)].T,
                      Wk[_neox_perm(HKV, swap=True)].T]).astype(bf)
    woT = Wo.T.astype(bf)
    wvp = np.zeros((577, 195), np.float32)
    for g in range(HKV):
        wvp[:C, 65 * g:65 * g + 64] = Wv[64 * g:64 * g + 64].T
        wvp[576, 65 * g + 64] = 1.0
    wvp = wvp.astype(bf)
    iden = np.eye(128, dtype=np.float32).astype(bf)

    per_j = _static_consts()
    ones = np.ones((1, T), np.float32)
    in_maps = []
    core_meta = []
    for b in range(B):
        xbT = x[b].T
        for j in range(2):
            keypos, qsel, c2k, s2k, masks = per_j[j]
            xkT = np.vstack([xbT[:, keypos], ones]).astype(bf)
            in_maps.append({
                "xkT": xkT,
                "wqqs": wqqs, "wkks": wkks, "wvp": wvp, "woT": woT,
                "c2k": c2k, "s2k": s2k,
                "masks": masks, "iden": iden,
            })
            core_meta.append((b, qsel))
    return in_maps, core_meta


def kernel(x, Wq, Wk, Wv, Wo):
    x = np.asarray(x, np.float32)
    Wq = np.asarray(Wq, np.float32)
    Wk = np.asarray(Wk, np.float32)
    Wv = np.asarray(Wv, np.float32)
    Wo = np.asarray(Wo, np.float32)

    from concourse.bass_utils import run_bass_kernel_spmd

    nc = _get_program()
    in_maps, core_meta = _host_prep(x, Wq, Wk, Wv, Wo)
    res = run_bass_kernel_spmd(nc, in_maps, list(range(8)))

    out = np.empty((B, T, C), np.float32)
    for core, (b, qsel) in enumerate(core_meta):
        out[b, qsel, :] = res.results[core]["yT"].T
    return out


# revision 8
# speedup vs baseline: 1.4622x; 1.0186x over previous
"""Trainium2 Bass kernel for CausalSelfAttention (RoPE + GQA), 8-core SPMD.

Sharding: 8 cores = 4 batches x 2 query-halves (as v1). Keys PERMUTED per
core so slot s consumes the static key-chunk range [2s, 2s+PAD_s); the first
1024 permuted keys ARE the core's queries, so Q projection reuses the same
x input and RoPE tables.

v2 changes vs v1 (236us):
  - All projections in bf16 (PE cost is free-dim rows only; bf16 allows
    free<256 at full rate and halves DMA). RoPE still via double projection
    (normal + pair-swapped weights), but packed into single weight matrices
    (Q+Qs = 1152 = 9x128 cols, K+Ks = 384 = 3x128) so no 64-row matmuls.
  - PV transposed: out[q,65] = P_chunk^T @ V[keys,65] with bf16 V moving
    (free 65 vs 256 -> PV PE cost halved); ones-column gives the softmax
    denominator per q-PARTITION, so the divide is a per-partition scalar op
    (gpsimd normalize_recip) instead of reciprocal+partition_broadcast+mul.
  - y [q, feat] transposed back for the output projection with PE bf16
    transposes via an identity matrix (cheap: 128 rows each).
  - exp -> bf16 P; mask multiply all-bf16 on DVE (2x mode).
  - Projection/transpose/oproj work is emitted through a filler queue
    interleaved between attention heads so PE fills the gaps of the
    ACT(exp)-paced attention stream.
"""
import sys

sys.path.insert(0, "/opt/trn_rl_repo")

import numpy as np
import ml_dtypes

B, T, C = 4, 2048, 576
H, HKV, D = 9, 3, 64
THETA = 10000.0
QB = 256                      # query block
TQ = 1024                     # queries per core
SLOT_PAD = [16, 12, 8, 4]     # padded key-chunk counts per slot
QBLOCKS = [[7, 5, 2, 0], [6, 4, 3, 1]]   # q-256-block ids per half j
KEYORDER = [[7, 5, 2, 0, 1, 3, 4, 6], [6, 4, 3, 1, 0, 2, 5, 7]]
CCX = [(0, 128), (128, 128), (256, 128), (384, 128), (512, 65)]   # 577 rows incl ones
CCQ = [(0, 128), (128, 128), (256, 128), (384, 128), (512, 64)]   # 576-row chunks
MM = [(0, 128), (128, 128), (256, 128), (384, 128), (512, 64)]    # 576 out chunks


def _slot_seq(s):
    """Key-chunk emission order for slot s: fulls, then the two diag chunks."""
    return list(range(2 * s + 2, 2 * s + SLOT_PAD[s])) + [2 * s, 2 * s + 1]


_PROG = None


def _build_program():
    import concourse.bacc as bacc
    import concourse.mybir as mybir
    import concourse.tile as tile

    dt = mybir.dt
    f32, bf16 = dt.float32, dt.bfloat16
    AF = mybir.ActivationFunctionType

    nc = bacc.Bacc("TRN2", target_bir_lowering=False, debug=False, num_devices=8)

    def inp(name, shape, d):
        return nc.declare_dram_parameter(name, shape, d, isOutput=False)

    xkT = inp("xkT", [577, T], bf16)
    wqqs = inp("wqqs", [C, 2 * C], bf16)
    wkks = inp("wkks", [C, 2 * HKV * D], bf16)
    wvp = inp("wvp", [577, 195], bf16)
    woT = inp("woT", [C, C], bf16)
    c2k = inp("c2k", [128, T], bf16)
    s2k = inp("s2k", [128, T], bf16)
    masksp = inp("masks", [128, 16 * QB], bf16)
    idenp = inp("iden", [128, 128], bf16)
    yT = nc.declare_dram_parameter("yT", [C, TQ], f32, isOutput=True)

    with tile.TileContext(nc) as tc:
        with (
            tc.tile_pool(name="const", bufs=1) as cp,
            tc.tile_pool(name="rope", bufs=2) as rp,
            tc.tile_pool(name="pwork", bufs=3) as pw,
            tc.tile_pool(name="ysb", bufs=2) as ysbp,
            tc.tile_pool(name="ost", bufs=2) as ostp,
            tc.tile_pool(name="psS", bufs=2, space="PSUM") as psS,
            tc.tile_pool(name="psY", bufs=2, space="PSUM") as psY,
        ):
            # ---------------- persistent constants ----------------
            # (DMA emission for most constants is deferred into the
            # projection phase so the first window's x/wk loads go first.)
            wo_r = [cp.tile([128, C], bf16, tag=f"wo{i}", name=f"wo{i}")
                    for i in range(5)]
            m_b = cp.tile([128, 16 * QB], bf16, tag="masks", name="masks")
            c2k_t = cp.tile([128, T], bf16, tag="c2k", name="c2k")
            s2k_t = cp.tile([128, T], bf16, tag="s2k", name="s2k")
            id_t = cp.tile([128, 128], bf16, tag="iden", name="iden")

            kt_h = [cp.tile([64, T], bf16, tag=f"kt{g}", name=f"kt{g}")
                    for g in range(HKV)]
            qth = [cp.tile([64, TQ], bf16, tag=f"qth{h}", name=f"qth{h}")
                   for h in range(H)]
            v_t = [cp.tile([128, 195], bf16, tag=f"v{c}", name=f"v{c}")
                   for c in range(16)]
            yq = [cp.tile([128, C], bf16, tag=f"yq{q}", name=f"yq{q}")
                  for q in range(8)]
            ypr = [cp.tile([128, TQ], bf16, tag=f"ypr{p}", name=f"ypr{p}")
                   for p in range(5)]

            # ---------------- attention ----------------
            fillers = []

            def pump(k=1):
                for _ in range(k):
                    if fillers:
                        fillers.pop(0)()

            def attn_slot(s, after_head=None):
                seq = _slot_seq(s)
                n = len(seq)
                for h in range(H):
                    g = h // 3
                    yh = [psY.tile([128, 65], f32, tag="ypsum", name="ypsum",
                                   padded_shape=[128, 512]) for _ in range(2)]
                    for sc in range(n // 4):
                        sp = psS.tile([128, 4 * QB], f32, tag="scores",
                                      name="scores")
                        for i in range(4):
                            c = seq[4 * sc + i]
                            nc.tensor.matmul(
                                sp[:, QB * i:QB * (i + 1)],
                                kt_h[g][0:64, 128 * c:128 * (c + 1)],
                                qth[h][0:64, QB * s:QB * (s + 1)],
                                start=True, stop=True)
                        p_b = pw.tile([128, 4 * QB], bf16, tag="p", name="p")
                        nc.scalar.activation(p_b[:], sp[:], AF.Exp, scale=0.125)
                        if sc == n // 4 - 1:
                            nc.vector.tensor_mul(
                                p_b[:], p_b[:],
                                m_b[:, 1024 * s:1024 * (s + 1)])
                        for i in range(4):
                            c = seq[4 * sc + i]
                            ci = 4 * sc + i
                            for hf in range(2):
                                nc.tensor.matmul(
                                    yh[hf][:, 0:65],
                                    p_b[:, QB * i + 128 * hf:
                                        QB * i + 128 * hf + 128],
                                    v_t[c][:, 65 * g:65 * g + 65],
                                    start=(ci == 0), stop=(ci == n - 1))
                    for hf in range(2):
                        ys = ysbp.tile([128, 65], f32, tag="ysb", name="ysb")
                        nc.vector.tensor_copy(ys[:], yh[hf][:])
                        nc.gpsimd.normalize_recip(
                            yq[2 * s + hf][:, 64 * h:64 * h + 64],
                            ys[:, 0:64], ys[:, 64:65])
                    if after_head is not None:
                        after_head(h)
                    pump()

            # ---------------- projections (phase 1+2) ----------------
            with (
                tc.tile_pool(name="wp", bufs=1) as wp,
                tc.tile_pool(name="psA", bufs=1, space="PSUM") as psA,
                tc.tile_pool(name="psB", bufs=1, space="PSUM") as psB,
                tc.tile_pool(name="xk", bufs=2) as xkp,
            ):
                def load_w(param, chunks, cols, tag):
                    tiles = []
                    for i, (k0, kl) in enumerate(chunks):
                        t = wp.tile([128, cols], bf16, tag=f"{tag}{i}",
                                    name=f"{tag}{i}")
                        nc.sync.dma_start(t[:kl, :], param[k0:k0 + kl, :])
                        tiles.append(t)
                    return tiles

                def load_x(win):
                    xk_r = []
                    for i, (k0, kl) in enumerate(CCX):
                        t = xkp.tile([128, 512], bf16, tag=f"xk{i}",
                                     name=f"xk{i}")
                        nc.sync.dma_start(
                            t[:kl, :], xkT[k0:k0 + kl, 512 * win:512 * (win + 1)])
                        xk_r.append(t)
                    return xk_r

                pjc = [0]

                def pj(w_r, mi, xk_r):
                    # alternate psA/psB per emitted group so bufs=1 reuse is
                    # hidden behind the intervening group's matmuls
                    pool, tag = ((psA, "pja") if pjc[0] % 2 == 0
                                 else (psB, "pjb"))
                    pjc[0] += 1
                    ps = pool.tile([128, 512], f32, tag=tag, name=tag)
                    for ci, (k0, kl) in enumerate(CCQ):
                        nc.tensor.matmul(
                            ps[:, :],
                            w_r[ci][:kl, 128 * mi:128 * (mi + 1)],
                            xk_r[ci][:kl, :],
                            start=(ci == 0), stop=(ci == 4))
                    return ps

                def kproj_a(win, xk_r, st):
                    # wkks cols: [K g0,g1 | K g2, Ks g0 | Ks g1, Ks g2].
                    # Swapped-side muls are written cross-base so each add's
                    # two inputs share a base partition (verifier rule).
                    c0 = 512 * win
                    ps0 = pj(wk_r, 0, xk_r)
                    t1a = rp.tile([128, 512], bf16, tag="kt1a", name="kt1a")
                    nc.vector.tensor_mul(t1a[:], ps0[:], c2k_t[:, c0:c0 + 512])
                    ps1 = pj(wk_r, 1, xk_r)
                    t1b = rp.tile([64, 512], bf16, tag="kt1b", name="kt1b")
                    t2b = rp.tile([64, 512], bf16, tag="kt2b", name="kt2b")
                    nc.vector.tensor_mul(t1b[:], ps1[0:64, :],
                                         c2k_t[0:64, c0:c0 + 512])
                    nc.vector.tensor_mul(t2b[0:64, :], ps1[64:128, :],
                                         s2k_t[64:128, c0:c0 + 512])
                    st.update(t1a=t1a, t1b=t1b, t2b=t2b)

                def kproj_b(win, xk_r, st):
                    c0 = 512 * win
                    ps2 = pj(wk_r, 2, xk_r)
                    t2a = rp.tile([128, 512], bf16, tag="kt2a", name="kt2a")
                    nc.vector.tensor_mul(t2a[64:128, :], ps2[0:64, :],
                                         s2k_t[0:64, c0:c0 + 512])
                    nc.vector.tensor_mul(t2a[0:64, :], ps2[64:128, :],
                                         s2k_t[64:128, c0:c0 + 512])
                    t1a, t1b, t2b = st["t1a"], st["t1b"], st["t2b"]
                    nc.gpsimd.tensor_add(kt_h[0][0:64, c0:c0 + 512],
                                         t1a[0:64, :], t2b[0:64, :])
                    nc.gpsimd.tensor_add(kt_h[1][0:64, c0:c0 + 512],
                                         t1a[64:128, :], t2a[64:128, :])
                    nc.gpsimd.tensor_add(kt_h[2][0:64, c0:c0 + 512],
                                         t1b[0:64, :], t2a[0:64, :])

                def vproj(win, xk_r, ti):
                    t_ = 4 * win + ti
                    pool, tag = ((psA, "pja") if pjc[0] % 2 == 0
                                 else (psB, "pjb"))
                    pjc[0] += 1
                    ps = pool.tile([128, 512], f32, tag=tag, name=tag)
                    for ci, (k0, kl) in enumerate(CCX):
                        nc.tensor.matmul(
                            ps[:, 0:195],
                            xk_r[ci][:kl, 128 * ti:128 * (ti + 1)],
                            wv_r[ci][:kl, :],
                            start=(ci == 0), stop=(ci == 4))
                    nc.vector.tensor_copy(v_t[t_][:], ps[:, 0:195])

                def qproj(win, xk_r, lo, hi):
                    # wqqs cols: [Q h0..h8 | Qs h0..h8]. Qs h sits at col
                    # 576+64h (opposite 64-parity to Q h), so swapped muls
                    # write cross-base to align each add's input pair.
                    c0 = 512 * win
                    t1, t2 = {}, {}
                    for mi in range(lo, hi):
                        ps = pj(wq_r, mi, xk_r)
                        if mi <= 3:
                            t = rp.tile([128, 512], bf16, tag=f"qt1_{mi}",
                                        name=f"qt1_{mi}")
                            nc.vector.tensor_mul(t[:], ps[:],
                                                 c2k_t[:, c0:c0 + 512])
                            t1[mi] = t
                        elif mi == 4:
                            ta = rp.tile([64, 512], bf16, tag="qt1_4",
                                         name="qt1_4")
                            tb = rp.tile([64, 512], bf16, tag="qt2_4",
                                         name="qt2_4")
                            nc.vector.tensor_mul(ta[:], ps[0:64, :],
                                                 c2k_t[0:64, c0:c0 + 512])
                            # Qs h0 at rows 64:128 -> base 0
                            nc.vector.tensor_mul(tb[0:64, :], ps[64:128, :],
                                                 s2k_t[64:128, c0:c0 + 512])
                            t1[4], t2[4] = ta, tb
                        else:
                            t = rp.tile([128, 512], bf16, tag=f"qt2_{mi}",
                                        name=f"qt2_{mi}")
                            # rows 0:64 hold Qs h(odd-src), cross-based
                            nc.vector.tensor_mul(t[64:128, :], ps[0:64, :],
                                                 s2k_t[0:64, c0:c0 + 512])
                            nc.vector.tensor_mul(t[0:64, :], ps[64:128, :],
                                                 s2k_t[64:128, c0:c0 + 512])
                            t2[mi] = t
                    return t1, t2

                def qrope(win, t1, t2, hlo=0, hhi=H):
                    c0 = 512 * win
                    for h in range(hlo, hhi):
                        bd = 64 * (h % 2)
                        a = t1[h // 2]
                        b = t2[(576 + 64 * h) // 128]
                        nc.gpsimd.tensor_add(
                            qth[h][0:64, c0:c0 + 512],
                            a[bd:bd + 64, :], b[bd:bd + 64, :])

                def full_win(win, with_q, xk_r=None):
                    if xk_r is None:
                        xk_r = load_x(win)
                    st = {}
                    kproj_a(win, xk_r, st)
                    vproj(win, xk_r, 0)
                    kproj_b(win, xk_r, st)
                    for ti in range(1, 4):
                        vproj(win, xk_r, ti)
                    if with_q:
                        t1, t2 = qproj(win, xk_r, 0, 9)
                        qrope(win, t1, t2)

                # DMA order: HWDGE generates one DMA per 625ns, so emit in
                # consumption order: per-ci (wk, xk-win1, wq) triplets first,
                # then tables, wv; bulky late constants after window 1.
                wk_r, xk1, wq_r = [], [], []
                for i, (k0, kl) in enumerate(CCQ):
                    t = wp.tile([128, 2 * HKV * D], bf16, tag=f"wk{i}",
                                name=f"wk{i}")
                    nc.sync.dma_start(t[:kl, :], wkks[k0:k0 + kl, :])
                    wk_r.append(t)
                    k0x, klx = CCX[i]
                    tx = xkp.tile([128, 512], bf16, tag=f"xk{i}", name=f"xk{i}")
                    nc.sync.dma_start(tx[:klx, :], xkT[k0x:k0x + klx, 512:1024])
                    xk1.append(tx)
                    tq = wp.tile([128, 2 * C], bf16, tag=f"wq{i}", name=f"wq{i}")
                    nc.sync.dma_start(tq[:kl, :], wqqs[k0:k0 + kl, :])
                    wq_r.append(tq)
                nc.sync.dma_start(c2k_t[:], c2k[:])
                nc.sync.dma_start(s2k_t[:], s2k[:])
                wv_r = load_w(wvp, CCX, 195, "wv")

                # window 1 with Q before V (wq arrives before wv; attention
                # needs V only from window 2 onward), then window 2.
                st1 = {}
                kproj_a(1, xk1, st1)
                t1q, t2q = qproj(1, xk1, 0, 1)
                kproj_b(1, xk1, st1)
                t1b, t2b_ = qproj(1, xk1, 1, 9)
                t1q.update(t1b); t2q.update(t2b_)
                qrope(1, t1q, t2q)
                for ti in range(4):
                    vproj(1, xk1, ti)
                nc.sync.dma_start(m_b[:], masksp[:])
                nc.sync.dma_start(id_t[:], idenp[:])
                for i, (k0, kl) in enumerate(MM):
                    nc.sync.dma_start(wo_r[i][:kl, :], woT[k0:k0 + kl, :])
                full_win(2, False)
                del st1, t1q, t2q, t1b, t2b_

                def win_filler(win, with_q):
                    st = {}

                    def f_load():
                        st["xk"] = load_x(win)

                    def f_ka():
                        kproj_a(win, st["xk"], st)

                    def f_kb():
                        kproj_b(win, st["xk"], st)

                    def f_v(ti):
                        return lambda: vproj(win, st["xk"], ti)

                    def f_q(lo, hi):
                        def g():
                            t1, t2 = qproj(win, st["xk"], lo, hi)
                            st.setdefault("t1", {}).update(t1)
                            st.setdefault("t2", {}).update(t2)
                        return g

                    def f_qrope(hlo, hhi):
                        def g():
                            qrope(win, st["t1"], st["t2"], hlo, hhi)
                        return g

                    units = [f_load, f_ka, f_v(0), f_kb,
                             f_v(1), f_v(2), f_v(3)]
                    if with_q:
                        units += [f_q(0, 3), f_q(3, 6), f_q(6, 9),
                                  f_qrope(0, 5), f_qrope(5, 9)]
                    return units

                fillers.extend(win_filler(0, True))
                fillers.extend(win_filler(3, False))

                attn_slot(3)
                attn_slot(2)
                while fillers:
                    pump()
                attn_slot(1)

            # ---------------- transposes, out-proj, last slot ----------------
            with (
                tc.tile_pool(name="psT", bufs=1, space="PSUM") as psT,
                tc.tile_pool(name="psR", bufs=1, space="PSUM") as psR,
            ):
                def transp(qc):
                    def g():
                        for m, (mc0, mrows) in enumerate(MM):
                            pt = psT.tile([128, 128], bf16, tag="pt", name="pt",
                                          padded_shape=[128, 1024])
                            nc.tensor.matmul(pt[:mrows, :],
                                             yq[qc][:, mc0:mc0 + mrows],
                                             id_t[:], start=True, stop=True,
                                             is_transpose=True)
                            nc.vector.tensor_copy(
                                ypr[m][0:mrows, 128 * qc:128 * (qc + 1)],
                                pt[:mrows, :])
                    return g

                def oproj_m(qq, m, pool=None):
                    # qq = 256-col q-quarter; quarter 2s/2s+1 ready as soon
                    # as slot s is transposed, so most of oproj hides inside
                    # the remaining attention stream. The tail quarter
                    # ping-pongs psR with the (idle by then) psS pool.
                    def g():
                        mc0, mrows = MM[m]
                        po = pool if pool is not None else psR
                        tg = "scores" if po is psS else "pjr"
                        shp = [128, 1024] if po is psS else [128, 256]
                        ps = po.tile(shp, f32, tag=tg, name=tg,
                                     padded_shape=[128, 512] if po is psR
                                     else None)
                        for p, (pc0, pl) in enumerate(MM):
                            nc.tensor.matmul(
                                ps[:mrows, 0:256],
                                wo_r[p][:pl, mc0:mc0 + mrows],
                                ypr[p][:pl, 256 * qq:256 * (qq + 1)],
                                start=(p == 0), stop=(p == 4))
                        ost = ostp.tile([128, 256], f32, tag="ostage",
                                        name="ostage")
                        nc.vector.tensor_copy(ost[:mrows, :], ps[:mrows, 0:256])
                        nc.sync.dma_start(
                            yT[mc0:mc0 + mrows, 256 * qq:256 * (qq + 1)],
                            ost[:mrows, :])
                    return g

                fillers.extend([transp(6), transp(7)])
                fillers.extend([oproj_m(3, m) for m in range(5)])
                fillers.extend([transp(4), transp(5)])
                fillers.extend([oproj_m(2, m) for m in range(5)])
                fillers.extend([transp(2), transp(3)])
                fillers.extend([oproj_m(1, m) for m in range(5)])

                def transp_block(qc, m):
                    def g():
                        mc0, mrows = MM[m]
                        pt = psT.tile([128, 128], bf16, tag="pt", name="pt",
                                      padded_shape=[128, 1024])
                        nc.tensor.matmul(pt[:mrows, :],
                                         yq[qc][:, mc0:mc0 + mrows],
                                         id_t[:], start=True, stop=True,
                                         is_transpose=True)
                        nc.vector.tensor_copy(
                            ypr[m][0:mrows, 128 * qc:128 * (qc + 1)],
                            pt[:mrows, :])
                    return g

                def slot0_hook(h):
                    # yq[0]/yq[1] cols for feature-chunk m complete once
                    # heads 2m and 2m+1 have divided; transpose them now so
                    # only the m=4 block and oproj remain after the slot.
                    if h % 2 == 1 and h >= 1:
                        m = (h - 1) // 2
                        fillers.append(transp_block(0, m))
                        fillers.append(transp_block(1, m))

                attn_slot(0, after_head=slot0_hook)
                while fillers:
                    pump()
                transp_block(0, 4)()
                transp_block(1, 4)()
                for m in range(5):
                    oproj_m(0, m, psS if m % 2 == 1 else psR)()

    nc.compile()
    return nc


def _get_program():
    global _PROG
    if _PROG is None:
        _PROG = _build_program()
    return _PROG


def _neox_perm(nheads, swap=False):
    p = []
    for h in range(nheads):
        ev = [64 * h + 2 * j for j in range(32)]
        od = [64 * h + 2 * j + 1 for j in range(32)]
        p += (od + ev) if swap else (ev + od)
    return np.array(p)


_CONSTS = None


def _static_consts():
    """Input-independent per-core constants (tables, masks, key orders)."""
    global _CONSTS
    if _CONSTS is not None:
        return _CONSTS
    invf = THETA ** (-np.arange(32, dtype=np.float64) / 32)

    def tables(pos):
        ang = pos[None, :] * invf[:, None]
        cos, sin = np.cos(ang), np.sin(ang)
        c2 = np.tile(cos, (4, 1)).astype(np.float32)
        s2 = np.tile(np.vstack([-sin, sin]), (2, 1)).astype(np.float32)
        return c2, s2

    per_j = []
    for j in range(2):
        keypos = np.concatenate(
            [np.arange(QB * q, QB * (q + 1)) for q in KEYORDER[j]])
        qsel = keypos[:TQ]          # queries = first 1024 permuted keys
        c2k, s2k = tables(keypos.astype(np.float64))
        masks = np.zeros((16 * 128, QB), np.float32)
        for s in range(4):
            seq = _slot_seq(s)
            qpos = keypos[QB * s:QB * (s + 1)]
            for k in range(4):
                c = seq[-4 + k]
                kpos = keypos[128 * c:128 * (c + 1)]
                masks[(4 * s + k) * 128:(4 * s + k + 1) * 128] = (
                    kpos[:, None] <= qpos[None, :]).astype(np.float32)
        # device layout: [128, 16*QB] (16 chunk-masks side by side)
        masks2 = masks.reshape(16, 128, QB).transpose(1, 0, 2).reshape(128, 16 * QB)
        per_j.append((keypos, qsel,
                      c2k.astype(ml_dtypes.bfloat16),
                      s2k.astype(ml_dtypes.bfloat16),
                      masks2.astype(ml_dtypes.bfloat16)))
    _CONSTS = per_j
    return _CONSTS


def _host_prep(x, Wq, Wk, Wv, Wo):
    bf = ml_dtypes.bfloat16
    wqqs = np.hstack([Wq[_neox_perm(H)].T,
                      Wq[_neox_perm(H, swap=True)].T]).astype(bf)
    wkks = np.hstack([Wk[_neox_perm(HKV)].T,
                      Wk[_neox_perm(HKV, swap=True)].T]).astype(bf)
    woT = Wo.T.astype(bf)
    wvp = np.zeros((577, 195), np.float32)
    for g in range(HKV):
        wvp[:C, 65 * g:65 * g + 64] = Wv[64 * g:64 * g + 64].T
        wvp[576, 65 * g + 64] = 1.0
    wvp = wvp.astype(bf)
    iden = np.eye(128, dtype=np.float32).astype(bf)

    per_j = _static_consts()
    ones = np.ones((1, T), np.float32)
    in_maps = []
    core_meta = []
    for b in range(B):
        xbT = x[b].T
        for j in range(2):
            keypos, qsel, c2k, s2k, masks = per_j[j]
            xkT = np.vstack([xbT[:, keypos], ones]).astype(bf)
            in_maps.append({
                "xkT": xkT,
                "wqqs": wqqs, "wkks": wkks, "wvp": wvp, "woT": woT,
                "c2k": c2k, "s2k": s2k,
                "masks": masks, "iden": iden,
            })
            core_meta.append((b, qsel))
    return in_maps, core_meta


def kernel(x, Wq, Wk, Wv, Wo):
    x = np.asarray(x, np.float32)
    Wq = np.asarray(Wq, np.float32)
    Wk = np.asarray(Wk, np.float32)
    Wv = np.asarray(Wv, np.float32)
    Wo = np.asarray(Wo, np.float32)

    from concourse.bass_utils import run_bass_kernel_spmd

    nc = _get_program()
    in_maps, core_meta = _host_prep(x, Wq, Wk, Wv, Wo)
    res = run_bass_kernel_spmd(nc, in_maps, list(range(8)))

    out = np.empty((B, T, C), np.float32)
    for core, (b, qsel) in enumerate(core_meta):
        out[b, qsel, :] = res.results[core]["yT"].T
    return out
